# revision 5
# baseline (speedup 1.0000x reference)
"""Bass/Trainium2 kernel for nn_CapsuleLayer (dynamic routing capsule layer).

Reference computation:
    inputs: [B=32, J=2048, I=64], W: [K=32, J=2048, D=32, I=64]
    inputs_hat[b,k,j,d] = sum_i inputs[b,j,i] * W[k,j,d,i]
    3 routing iterations (softmax over K), output = squash(s_2)  [B, K, D]

Sharding: J (input capsules) split 8 ways -> J_loc = 256 per core.
Routing softmax (over K) is fully local; only the per-iteration
s[b,k,d] = sum_j c*hat partial sums need a 128KB AllReduce.

Device layouts (per core):
  x stations : [NPAIR=128, 128, 64]  fp16, block-diag pairs (2 j per station)
  W moving   : [NPAIR=128, 128, 1024] fp16 = [pair, (jp,i), (d,k)]
  hat        : SBUF fp16 [128, 64, 1024] = [(jj,b), group, (d,k)]
  s / outputs: [32, 1024] fp32 = [b, (d,k)]
"""

import os
import sys
import numpy as np

import concourse.bass as bass
import concourse.mybir as mybir
import concourse.tile as tile
from concourse import bacc
from concourse import bass_utils

AF = mybir.ActivationFunctionType
ALU = mybir.AluOpType
F16 = mybir.dt.float16
F32 = mybir.dt.float32

EPS = 1e-07
N_CORES = 8
B = 32          # batch
J = 2048        # input capsules (total)
I = 64          # input capsule dim
K = 32          # output capsules
D = 32          # output capsule dim
JL = J // N_CORES          # 256 local input capsules
NPAIR = JL // 2            # 128 station pairs
NGRP = JL // 4             # 64 groups of 4 j's
GPC = 4                    # groups per chunk in routing passes
NCHUNK = NGRP // GPC       # 16 chunks
DK = D * K                 # 1024


def build_program():
    """Build the SPMD bass program (same program on all 8 cores)."""
    nc = bacc.Bacc("TRN2", target_bir_lowering=False, debug=False,
                   enable_asserts=False, num_devices=N_CORES)

    xs = nc.dram_tensor("xs", [NPAIR, 128, I], F16, kind="ExternalInput").ap()
    wt = nc.dram_tensor("wt", [NPAIR, 128, DK], F16, kind="ExternalInput").ap()
    diag = nc.dram_tensor("diag", [128, B], F16, kind="ExternalInput").ap()
    out_d = nc.dram_tensor("out", [B, DK], F32, kind="ExternalOutput").ap()

    with tile.TileContext(nc) as tc:
        _emit(tc, xs, wt, diag, out_d)
    nc.compile()
    return nc


def _emit(tc, xs, wt, diag, out_d):
    nc = tc.nc
    with (
        tc.tile_pool(name="hat", bufs=1) as hat_pool,
        tc.tile_pool(name="wld", bufs=2) as w_pool,
        tc.tile_pool(name="xld", bufs=2) as x_pool,
        tc.tile_pool(name="big", bufs=2) as big_pool,       # prod/ch chunk tiles
        tc.tile_pool(name="tree", bufs=1) as tree_pool,
        tc.tile_pool(name="smx", bufs=1) as smx_pool,
        tc.tile_pool(name="small", bufs=1) as small_pool,
        tc.tile_pool(name="obc", bufs=1) as obc_pool,
        tc.tile_pool(name="const", bufs=1) as const_pool,
        tc.tile_pool(name="accps", bufs=1, space="PSUM") as acc_psum,
        tc.tile_pool(name="hatps", bufs=2, space="PSUM") as hat_psum,
        tc.tile_pool(name="dram", bufs=6, space="DRAM") as dram_pool,
    ):
        # ---- constants ----
        diag_sb = const_pool.tile([128, B], F16, tag="diag")
        nc.sync.dma_start(diag_sb[:], diag)

        # persistent hat storage: [(jj,b), group, (d,k)] fp16
        hat_sb = hat_pool.tile([128, NGRP, DK], F16, tag="hat")

        # O accumulator (sum of squash outputs over past iterations)
        o_acc = const_pool.tile([B, DK], F32, tag="oacc")

        # ---- Pass A: hat = x @ W, and s0 = sum_j hat (PSUM accumulate) ----
        s_ps = acc_psum.tile([B, DK], F32, tag="sacc", name="s0_ps")
        for g in range(NGRP):
            wg = w_pool.tile([128, 2, DK], F16, tag="w")
            nc.sync.dma_start(wg[:], wt[2 * g: 2 * g + 2].rearrange("q p f -> p q f"))
            xg = x_pool.tile([128, 2, I], F16, tag="x")
            nc.sync.dma_start(xg[:], xs[2 * g: 2 * g + 2].rearrange("q p f -> p q f"))

            ps = hat_psum.tile([128, DK], F32, tag="hatps", name=f"hat_ps{g}")
            for q in (0, 1):            # station pair within group
                for h in (0, 1):        # free-dim half
                    nc.tensor.matmul(
                        ps[q * 64:(q + 1) * 64, h * 512:(h + 1) * 512],
                        lhsT=xg[:, q, :],
                        rhs=wg[:, q, h * 512:(h + 1) * 512],
                        start=True, stop=True,
                        tile_position=(0, q * 64),
                    )
            # PSUM -> SBUF fp16 (split across ScalarE / VectorE)
            nc.scalar.copy(hat_sb[:, g, 0:512], ps[:, 0:512])
            nc.vector.tensor_copy(hat_sb[:, g, 512:DK], ps[:, 512:DK])
            # s0 accumulation: sum over (jj) partitions via diag stationary
            for h in (0, 1):
                nc.tensor.matmul(
                    s_ps[:, h * 512:(h + 1) * 512],
                    lhsT=diag_sb[:],
                    rhs=hat_sb[:, g, h * 512:(h + 1) * 512],
                    start=(g == 0), stop=(g == NGRP - 1),
                )

        # ---- routing iterations ----
        for r in range(3):
            # s partial -> AllReduce -> s_full
            s_loc = small_pool.tile([B, DK], F32, tag="sloc", name=f"s_loc{r}")
            nc.vector.tensor_copy(s_loc[:], s_ps[:])
            ar_in = dram_pool.tile([B, DK], F32, name=f"ar_in{r}")
            ar_out = dram_pool.tile([B, DK], F32, name=f"ar_out{r}")
            nc.sync.dma_start(ar_in[:], s_loc[:])
            nc.gpsimd.collective_compute(
                "AllReduce", ALU.add,
                replica_groups=[list(range(N_CORES))],
                ins=[ar_in.opt()],
                outs=[ar_out.opt()],
            )
            s_full = small_pool.tile([B, DK], F32, tag="sfull", name=f"s_full{r}")
            nc.sync.dma_start(s_full[:], ar_out[:])
            if r == 0:
                nc.vector.tensor_scalar_mul(s_full[:], s_full[:], 1.0 / K)

            # squash: scale = s2/(1+s2)/sqrt(s2+eps), per (b,k); s2 = sum_d s^2
            sq = small_pool.tile([B, DK], F32, tag="sq")
            nc.scalar.square(sq[:], s_full[:])
            s2 = small_pool.tile([B, K], F32, tag="s2")
            nc.vector.reduce_sum(s2[:], sq.rearrange("p (d k) -> p k d", d=D),
                                 axis=mybir.AxisListType.X)
            t1 = small_pool.tile([B, K], F32, tag="t1")
            nc.vector.tensor_scalar_add(t1[:], s2[:], 1.0)
            t2 = small_pool.tile([B, K], F32, tag="t2")
            nc.vector.tensor_scalar_add(t2[:], s2[:], EPS)
            nc.scalar.sqrt(t2[:], t2[:])
            nc.vector.tensor_mul(t1[:], t1[:], t2[:])         # (1+s2)*sqrt(s2+eps)
            nc.vector.reciprocal(t1[:], t1[:])
            nc.vector.tensor_mul(s2[:], s2[:], t1[:])         # scale [B, K]
            o_r = small_pool.tile([B, DK], F32, tag="or", name=f"o_{r}")
            nc.vector.tensor_tensor(
                o_r.rearrange("p (d k) -> p d k", d=D),
                s_full.rearrange("p (d k) -> p d k", d=D),
                s2[:, None, :].to_broadcast([B, D, K]),
                ALU.mult,
            )

            if r == 2:
                nc.sync.dma_start(out_d, o_r[:])
                break

            # O_acc += o_r ; build O_bcast fp16 [128, (d,k)]
            if r == 0:
                nc.vector.tensor_copy(o_acc[:], o_r[:])
            else:
                nc.vector.tensor_add(o_acc[:], o_acc[:], o_r[:])
            o16 = small_pool.tile([B, DK], F16, tag="o16", name=f"o16_{r}")
            nc.vector.tensor_copy(o16[:], o_acc[:])
            o_bc = obc_pool.tile([128, DK], F16, tag="obc", name=f"obc_{r}")
            for jj in range(4):
                nc.sync.dma_start(o_bc[jj * 32:(jj + 1) * 32, :], o16[:])

            # next-iteration s accumulator
            s_ps = acc_psum.tile([B, DK], F32, tag="sacc", name=f"s{r + 1}_ps")

            # routing pass over hat chunks
            for ci in range(NCHUNK):
                gsl = slice(ci * GPC, (ci + 1) * GPC)
                hat_c = hat_sb[:, gsl, :]
                # u = sum_d hat * O_acc   (fp16 mul + pairwise tree over d)
                prod = big_pool.tile([128, GPC, DK], F16, tag="big",
                                     name=f"prod_{r}_{ci}")
                nc.vector.tensor_tensor(
                    prod[:], hat_c,
                    o_bc[:, None, :].to_broadcast([128, GPC, DK]),
                    ALU.mult,
                )
                p4 = prod.rearrange("p g (d k) -> p g d k", d=D)
                t16 = tree_pool.tile([128, GPC, 16, K], F16, tag="t16")
                nc.vector.tensor_add(t16[:], p4[:, :, 0:16, :], p4[:, :, 16:32, :])
                t8 = tree_pool.tile([128, GPC, 8, K], F16, tag="t8")
                nc.vector.tensor_add(t8[:], t16[:, :, 0:8, :], t16[:, :, 8:16, :])
                t4 = tree_pool.tile([128, GPC, 4, K], F16, tag="t4")
                nc.vector.tensor_add(t4[:], t8[:, :, 0:4, :], t8[:, :, 4:8, :])
                t2t = tree_pool.tile([128, GPC, 2, K], F32, tag="t2")
                nc.vector.tensor_add(t2t[:], t4[:, :, 0:2, :], t4[:, :, 2:4, :])
                u = smx_pool.tile([128, GPC, K], F32, tag="u")
                nc.vector.tensor_add(u[:], t2t[:, :, 0, :], t2t[:, :, 1, :])

                # softmax over k (free dim)
                umax = smx_pool.tile([128, GPC], F32, tag="umax")
                nc.vector.reduce_max(umax[:], u[:], axis=mybir.AxisListType.X)
                nc.vector.tensor_sub(u[:], u[:],
                                     umax[:, :, None].to_broadcast([128, GPC, K]))
                nc.scalar.activation(u[:], u[:], AF.Exp)
                z = smx_pool.tile([128, GPC], F32, tag="z")
                nc.vector.reduce_sum(z[:], u[:], axis=mybir.AxisListType.X)
                nc.vector.reciprocal(z[:], z[:])
                c16 = smx_pool.tile([128, GPC, K], F16, tag="c16")
                nc.vector.tensor_tensor(
                    c16[:], u[:], z[:, :, None].to_broadcast([128, GPC, K]),
                    ALU.mult,
                )

                # ch = c * hat ; PE partition-sum into s_ps
                ch = big_pool.tile([128, GPC, DK], F16, tag="big",
                                   name=f"ch_{r}_{ci}")
                nc.vector.tensor_tensor(
                    ch.rearrange("p g (d k) -> p g d k", d=D),
                    hat_c.rearrange("p g (d k) -> p g d k", d=D),
                    c16[:, :, None, :].to_broadcast([128, GPC, D, K]),
                    ALU.mult,
                )
                for gg in range(GPC):
                    for h in (0, 1):
                        nc.tensor.matmul(
                            s_ps[:, h * 512:(h + 1) * 512],
                            lhsT=diag_sb[:],
                            rhs=ch[:, gg, h * 512:(h + 1) * 512],
                            start=(ci == 0 and gg == 0),
                            stop=(ci == NCHUNK - 1 and gg == GPC - 1),
                        )


def pack_inputs(inputs, W):
    """Host-side shard + layout pack. Returns in_maps (one dict per core)."""
    diag = np.zeros((128, B), np.float16)
    for p in range(128):
        diag[p, p % B] = 1.0

    # W: [K, J, D, I] -> per core [JL, I, D, K] fp16 -> [NPAIR, 128, DK]
    in_maps = []
    for c in range(N_CORES):
        jsl = slice(c * JL, (c + 1) * JL)
        wc = np.ascontiguousarray(
            W[:, jsl].transpose(1, 3, 2, 0), dtype=np.float16
        )  # [JL, I, D, K]
        wt = wc.reshape(NPAIR, 2 * I, DK)

        xc = inputs[:, jsl, :]  # [B, JL, I]
        xs = np.zeros((NPAIR, 128, I), np.float16)
        xt = np.ascontiguousarray(xc.transpose(1, 2, 0))  # [JL, I, B]
        xs[:, 0:I, 0:B] = xt[0::2]
        xs[:, I:128, B:2 * B] = xt[1::2]
        in_maps.append({"xs": xs, "wt": wt, "diag": diag})
    return in_maps


_CACHED_NC = None


def _install_ntff_hook():
    """Provide antenv.axon_hooks.get_axon_ntff_profile_hook when the agent
    image lacks it, by driving the injected libaxon_pjrt.so directly
    (mirrors trn_agent_boot._ntff_profile_via_ctypes)."""
    import types
    import ctypes
    import contextlib
    try:
        from antenv.axon_hooks import get_axon_ntff_profile_hook  # noqa: F401
        return True
    except ImportError:
        pass
    so_path = "/opt/axon/libaxon_pjrt.so"
    if not os.path.exists(so_path):
        return False
    lib = ctypes.CDLL(so_path)
    if not hasattr(lib, "axon_start_nrt_profile"):
        return False
    lib.axon_start_nrt_profile.argtypes = [
        ctypes.POINTER(ctypes.c_int64), ctypes.c_size_t]
    lib.axon_start_nrt_profile.restype = ctypes.c_int64
    lib.axon_stop_nrt_profile.argtypes = [ctypes.c_char_p]
    lib.axon_stop_nrt_profile.restype = ctypes.c_int64

    @contextlib.contextmanager
    def _hook(output_dir, device_ids):
        import jax
        jax.devices()
        if device_ids:
            ids = (ctypes.c_int64 * len(device_ids))(*device_ids)
            rc = lib.axon_start_nrt_profile(ids, len(device_ids))
        else:
            rc = lib.axon_start_nrt_profile(None, 0)
        if rc != 0:
            raise RuntimeError(f"axon_start_nrt_profile rc={rc}")
        try:
            yield
        finally:
            n = lib.axon_stop_nrt_profile(str(output_dir).encode())
            if n < 0:
                raise RuntimeError(f"axon_stop_nrt_profile rc={n}")

    import antenv
    mod = types.ModuleType("antenv.axon_hooks")
    mod.get_axon_ntff_profile_hook = lambda: _hook
    mod.set_axon_ntff_profile_hook = lambda h: None
    sys.modules["antenv.axon_hooks"] = mod
    antenv.axon_hooks = mod
    return True


def kernel(inputs, W):
    global _CACHED_NC
    inputs = np.asarray(inputs)
    W = np.asarray(W)
    if _CACHED_NC is None:
        _CACHED_NC = build_program()
    nc = _CACHED_NC
    in_maps = pack_inputs(inputs, W)
    trace = bool(int(os.environ.get("CAPS_TRACE", "0")))
    if trace:
        trace = _install_ntff_hook()
    res = bass_utils.run_bass_kernel_spmd(
        nc, in_maps, core_ids=list(range(N_CORES)), trace=trace,
    )
    kernel.last_results = res
    if trace and res.exec_time_ns is not None:
        print(f"HW exec time: {res.exec_time_ns} ns", file=sys.stderr)
        kernel.last_exec_time_ns = res.exec_time_ns
    out = res.results[0]["out"]  # [B, DK] fp32, identical on all cores
    return np.ascontiguousarray(
        out.reshape(B, D, K).transpose(0, 2, 1)
    ).astype(np.float32)


kernel.last_exec_time_ns = None
kernel.last_results = None


# revision 8
# speedup vs baseline: 1.0155x; 1.0155x over previous
"""Bass/Trainium2 kernel for nn_CapsuleLayer (dynamic routing capsule layer).

Reference computation:
    inputs: [B=32, J=2048, I=64], W: [K=32, J=2048, D=32, I=64]
    inputs_hat[b,k,j,d] = sum_i inputs[b,j,i] * W[k,j,d,i]
    3 routing iterations (softmax over K), output = squash(s_2)  [B, K, D]

Sharding: J (input capsules) split 8 ways -> J_loc = 256 per core.
Routing softmax (over K) is fully local; only the per-iteration
s[b,k,d] = sum_j c*hat partial sums need a 128KB AllReduce.

Device layouts (per core):
  x stations : [NPAIR=128, 128, 64]  fp16, block-diag pairs (2 j per station)
  W moving   : [NPAIR=128, 128, 1024] fp16 = [pair, (jp,i), (d,k)]
  hat        : SBUF fp16 [128, 64, 1024] = [(jj,b), group, (d,k)]
  s / outputs: [32, 1024] fp32 = [b, (d,k)]
"""

import os
import sys
import numpy as np

import concourse.bass as bass
import concourse.mybir as mybir
import concourse.tile as tile
from concourse import bacc
from concourse import bass_utils

AF = mybir.ActivationFunctionType
ALU = mybir.AluOpType
F16 = mybir.dt.float16
F32 = mybir.dt.float32

EPS = 1e-07
N_CORES = 8
B = 32          # batch
J = 2048        # input capsules (total)
I = 64          # input capsule dim
K = 32          # output capsules
D = 32          # output capsule dim
JL = J // N_CORES          # 256 local input capsules
NPAIR = JL // 2            # 128 station pairs
NGRP = JL // 4             # 64 groups of 4 j's
GPC = 4                    # groups per chunk in routing passes
NCHUNK = NGRP // GPC       # 16 chunks
DK = D * K                 # 1024


def build_program():
    """Build the SPMD bass program (same program on all 8 cores)."""
    nc = bacc.Bacc("TRN2", target_bir_lowering=False, debug=False,
                   enable_asserts=False, num_devices=N_CORES)

    xs = nc.dram_tensor("xs", [NPAIR, 128, I], F16, kind="ExternalInput").ap()
    wt = nc.dram_tensor("wt", [NPAIR, 128, DK], F16, kind="ExternalInput").ap()
    diag = nc.dram_tensor("diag", [128, B], F16, kind="ExternalInput").ap()
    out_d = nc.dram_tensor("out", [B, DK], F32, kind="ExternalOutput").ap()

    with tile.TileContext(nc) as tc:
        _emit(tc, xs, wt, diag, out_d)
    nc.compile()
    return nc


def _emit(tc, xs, wt, diag, out_d):
    nc = tc.nc
    with (
        tc.tile_pool(name="hat", bufs=1) as hat_pool,
        tc.tile_pool(name="wld", bufs=2) as w_pool,
        tc.tile_pool(name="xld", bufs=2) as x_pool,
        tc.tile_pool(name="big", bufs=2) as big_pool,       # prod/ch chunk tiles
        tc.tile_pool(name="tree", bufs=1) as tree_pool,
        tc.tile_pool(name="smx", bufs=1) as smx_pool,
        tc.tile_pool(name="small", bufs=1) as small_pool,
        tc.tile_pool(name="obc", bufs=1) as obc_pool,
        tc.tile_pool(name="const", bufs=1) as const_pool,
        tc.tile_pool(name="accps", bufs=1, space="PSUM") as acc_psum,
        tc.tile_pool(name="hatps", bufs=2, space="PSUM") as hat_psum,
        tc.tile_pool(name="dram", bufs=6, space="DRAM") as dram_pool,
    ):
        # ---- constants ----
        diag_sb = const_pool.tile([128, B], F16, tag="diag")
        nc.sync.dma_start(diag_sb[:], diag)

        # persistent hat storage: [(jj,b), group, (d,k)] fp16
        hat_sb = hat_pool.tile([128, NGRP, DK], F16, tag="hat")

        # O accumulator (sum of squash outputs over past iterations)
        o_acc = const_pool.tile([B, DK], F32, tag="oacc")

        # ---- Pass A: hat = x @ W, and s0 = sum_j hat (PSUM accumulate) ----
        s_ps = acc_psum.tile([B, DK], F32, tag="sacc", name="s0_ps")
        for g in range(NGRP):
            wg = w_pool.tile([128, 2, DK], F16, tag="w")
            nc.sync.dma_start(wg[:], wt[2 * g: 2 * g + 2].rearrange("q p f -> p q f"))
            xg = x_pool.tile([128, 2, I], F16, tag="x")
            nc.sync.dma_start(xg[:], xs[2 * g: 2 * g + 2].rearrange("q p f -> p q f"))

            ps = hat_psum.tile([128, DK], F32, tag="hatps", name=f"hat_ps{g}")
            for q in (0, 1):            # station pair within group
                for h in (0, 1):        # free-dim half
                    nc.tensor.matmul(
                        ps[q * 64:(q + 1) * 64, h * 512:(h + 1) * 512],
                        lhsT=xg[:, q, :],
                        rhs=wg[:, q, h * 512:(h + 1) * 512],
                        start=True, stop=True,
                        tile_position=(0, q * 64),
                    )
            # PSUM -> SBUF fp16 (split across ScalarE / VectorE)
            nc.scalar.copy(hat_sb[:, g, 0:512], ps[:, 0:512])
            nc.vector.tensor_copy(hat_sb[:, g, 512:DK], ps[:, 512:DK])

        # s0 = sum_j hat: dense single-station MM block (keeps PE warm,
        # no per-group LDW interleave)
        for g in range(NGRP):
            for h in (0, 1):
                nc.tensor.matmul(
                    s_ps[:, h * 512:(h + 1) * 512],
                    lhsT=diag_sb[:],
                    rhs=hat_sb[:, g, h * 512:(h + 1) * 512],
                    start=(g == 0), stop=(g == NGRP - 1),
                )

        # ---- routing iterations ----
        for r in range(3):
            # s partial -> AllReduce -> s_full
            s_loc = small_pool.tile([B, DK], F32, tag="sloc", name=f"s_loc{r}")
            nc.vector.tensor_copy(s_loc[:], s_ps[:])
            ar_in = dram_pool.tile([B, DK], F32, name=f"ar_in{r}")
            ar_out = dram_pool.tile([B, DK], F32, name=f"ar_out{r}")
            nc.sync.dma_start(ar_in[:], s_loc[:])
            nc.gpsimd.collective_compute(
                "AllReduce", ALU.add,
                replica_groups=[list(range(N_CORES))],
                ins=[ar_in.opt()],
                outs=[ar_out.opt()],
            )
            s_full = small_pool.tile([B, DK], F32, tag="sfull", name=f"s_full{r}")
            nc.sync.dma_start(s_full[:], ar_out[:])
            if r == 0:
                nc.vector.tensor_scalar_mul(s_full[:], s_full[:], 1.0 / K)

            # squash: scale = s2/(1+s2)/sqrt(s2+eps), per (b,k); s2 = sum_d s^2
            sq = small_pool.tile([B, DK], F32, tag="sq")
            nc.scalar.square(sq[:], s_full[:])
            s2 = small_pool.tile([B, K], F32, tag="s2")
            nc.vector.reduce_sum(s2[:], sq.rearrange("p (d k) -> p k d", d=D),
                                 axis=mybir.AxisListType.X)
            t1 = small_pool.tile([B, K], F32, tag="t1")
            nc.vector.tensor_scalar_add(t1[:], s2[:], 1.0)
            t2 = small_pool.tile([B, K], F32, tag="t2")
            nc.vector.tensor_scalar_add(t2[:], s2[:], EPS)
            nc.scalar.sqrt(t2[:], t2[:])
            nc.vector.tensor_mul(t1[:], t1[:], t2[:])         # (1+s2)*sqrt(s2+eps)
            nc.vector.reciprocal(t1[:], t1[:])
            nc.vector.tensor_mul(s2[:], s2[:], t1[:])         # scale [B, K]
            o_r = small_pool.tile([B, DK], F32, tag="or", name=f"o_{r}")
            nc.vector.tensor_tensor(
                o_r.rearrange("p (d k) -> p d k", d=D),
                s_full.rearrange("p (d k) -> p d k", d=D),
                s2[:, None, :].to_broadcast([B, D, K]),
                ALU.mult,
            )

            if r == 2:
                nc.sync.dma_start(out_d, o_r[:])
                break

            # O_acc += o_r ; build O_bcast fp16 [128, (d,k)]
            if r == 0:
                nc.vector.tensor_copy(o_acc[:], o_r[:])
            else:
                nc.vector.tensor_add(o_acc[:], o_acc[:], o_r[:])
            o16 = small_pool.tile([B, DK], F16, tag="o16", name=f"o16_{r}")
            nc.vector.tensor_copy(o16[:], o_acc[:])
            o_bc = obc_pool.tile([128, DK], F16, tag="obc", name=f"obc_{r}")
            for jj in range(4):
                nc.sync.dma_start(o_bc[jj * 32:(jj + 1) * 32, :], o16[:])

            # next-iteration s accumulator
            s_ps = acc_psum.tile([B, DK], F32, tag="sacc", name=f"s{r + 1}_ps")

            # routing pass over hat chunks
            for ci in range(NCHUNK):
                gsl = slice(ci * GPC, (ci + 1) * GPC)
                hat_c = hat_sb[:, gsl, :]
                # u = sum_d hat * O_acc   (fp16 mul + pairwise tree over d)
                prod = big_pool.tile([128, GPC, DK], F16, tag="big",
                                     name=f"prod_{r}_{ci}")
                nc.vector.tensor_tensor(
                    prod[:], hat_c,
                    o_bc[:, None, :].to_broadcast([128, GPC, DK]),
                    ALU.mult,
                )
                p4 = prod.rearrange("p g (d k) -> p g d k", d=D)
                t16 = tree_pool.tile([128, GPC, 16, K], F16, tag="t16")
                nc.vector.tensor_add(t16[:], p4[:, :, 0:16, :], p4[:, :, 16:32, :])
                t8 = tree_pool.tile([128, GPC, 8, K], F16, tag="t8")
                nc.vector.tensor_add(t8[:], t16[:, :, 0:8, :], t16[:, :, 8:16, :])
                t4 = tree_pool.tile([128, GPC, 4, K], F16, tag="t4")
                nc.vector.tensor_add(t4[:], t8[:, :, 0:4, :], t8[:, :, 4:8, :])
                t2t = tree_pool.tile([128, GPC, 2, K], F32, tag="t2")
                nc.vector.tensor_add(t2t[:], t4[:, :, 0:2, :], t4[:, :, 2:4, :])
                u = smx_pool.tile([128, GPC, K], F32, tag="u")
                nc.vector.tensor_add(u[:], t2t[:, :, 0, :], t2t[:, :, 1, :])

                # softmax over k (free dim)
                umax = smx_pool.tile([128, GPC], F32, tag="umax")
                nc.vector.reduce_max(umax[:], u[:], axis=mybir.AxisListType.X)
                nc.vector.tensor_sub(u[:], u[:],
                                     umax[:, :, None].to_broadcast([128, GPC, K]))
                nc.scalar.activation(u[:], u[:], AF.Exp)
                z = smx_pool.tile([128, GPC], F32, tag="z")
                nc.vector.reduce_sum(z[:], u[:], axis=mybir.AxisListType.X)
                nc.vector.reciprocal(z[:], z[:])
                c16 = smx_pool.tile([128, GPC, K], F16, tag="c16")
                nc.vector.tensor_tensor(
                    c16[:], u[:], z[:, :, None].to_broadcast([128, GPC, K]),
                    ALU.mult,
                )

                # ch = c * hat ; PE partition-sum into s_ps
                ch = big_pool.tile([128, GPC, DK], F16, tag="big",
                                   name=f"ch_{r}_{ci}")
                nc.vector.tensor_tensor(
                    ch.rearrange("p g (d k) -> p g d k", d=D),
                    hat_c.rearrange("p g (d k) -> p g d k", d=D),
                    c16[:, :, None, :].to_broadcast([128, GPC, D, K]),
                    ALU.mult,
                )
                for gg in range(GPC):
                    for h in (0, 1):
                        nc.tensor.matmul(
                            s_ps[:, h * 512:(h + 1) * 512],
                            lhsT=diag_sb[:],
                            rhs=ch[:, gg, h * 512:(h + 1) * 512],
                            start=(ci == 0 and gg == 0),
                            stop=(ci == NCHUNK - 1 and gg == GPC - 1),
                        )


def pack_inputs(inputs, W):
    """Host-side shard + layout pack. Returns in_maps (one dict per core)."""
    diag = np.zeros((128, B), np.float16)
    for p in range(128):
        diag[p, p % B] = 1.0

    # W: [K, J, D, I] -> per core [JL, I, D, K] fp16 -> [NPAIR, 128, DK]
    in_maps = []
    for c in range(N_CORES):
        jsl = slice(c * JL, (c + 1) * JL)
        wc = np.ascontiguousarray(
            W[:, jsl].transpose(1, 3, 2, 0), dtype=np.float16
        )  # [JL, I, D, K]
        wt = wc.reshape(NPAIR, 2 * I, DK)

        xc = inputs[:, jsl, :]  # [B, JL, I]
        xs = np.zeros((NPAIR, 128, I), np.float16)
        xt = np.ascontiguousarray(xc.transpose(1, 2, 0))  # [JL, I, B]
        xs[:, 0:I, 0:B] = xt[0::2]
        xs[:, I:128, B:2 * B] = xt[1::2]
        in_maps.append({"xs": xs, "wt": wt, "diag": diag})
    return in_maps


_CACHED_NC = None


def _install_ntff_hook():
    """Provide antenv.axon_hooks.get_axon_ntff_profile_hook when the agent
    image lacks it, by driving the injected libaxon_pjrt.so directly
    (mirrors trn_agent_boot._ntff_profile_via_ctypes)."""
    import types
    import ctypes
    import contextlib
    try:
        from antenv.axon_hooks import get_axon_ntff_profile_hook  # noqa: F401
        return True
    except ImportError:
        pass
    so_path = "/opt/axon/libaxon_pjrt.so"
    if not os.path.exists(so_path):
        return False
    lib = ctypes.CDLL(so_path)
    if not hasattr(lib, "axon_start_nrt_profile"):
        return False
    lib.axon_start_nrt_profile.argtypes = [
        ctypes.POINTER(ctypes.c_int64), ctypes.c_size_t]
    lib.axon_start_nrt_profile.restype = ctypes.c_int64
    lib.axon_stop_nrt_profile.argtypes = [ctypes.c_char_p]
    lib.axon_stop_nrt_profile.restype = ctypes.c_int64

    @contextlib.contextmanager
    def _hook(output_dir, device_ids):
        import jax
        jax.devices()
        if device_ids:
            ids = (ctypes.c_int64 * len(device_ids))(*device_ids)
            rc = lib.axon_start_nrt_profile(ids, len(device_ids))
        else:
            rc = lib.axon_start_nrt_profile(None, 0)
        if rc != 0:
            raise RuntimeError(f"axon_start_nrt_profile rc={rc}")
        try:
            yield
        finally:
            n = lib.axon_stop_nrt_profile(str(output_dir).encode())
            if n < 0:
                raise RuntimeError(f"axon_stop_nrt_profile rc={n}")

    import antenv
    mod = types.ModuleType("antenv.axon_hooks")
    mod.get_axon_ntff_profile_hook = lambda: _hook
    mod.set_axon_ntff_profile_hook = lambda h: None
    sys.modules["antenv.axon_hooks"] = mod
    antenv.axon_hooks = mod
    return True


def kernel(inputs, W):
    global _CACHED_NC
    inputs = np.asarray(inputs)
    W = np.asarray(W)
    if _CACHED_NC is None:
        _CACHED_NC = build_program()
    nc = _CACHED_NC
    in_maps = pack_inputs(inputs, W)
    trace = bool(int(os.environ.get("CAPS_TRACE", "0")))
    if trace:
        trace = _install_ntff_hook()
    res = bass_utils.run_bass_kernel_spmd(
        nc, in_maps, core_ids=list(range(N_CORES)), trace=trace,
    )
    kernel.last_results = res
    if trace and res.exec_time_ns is not None:
        print(f"HW exec time: {res.exec_time_ns} ns", file=sys.stderr)
        kernel.last_exec_time_ns = res.exec_time_ns
    out = res.results[0]["out"]  # [B, DK] fp32, identical on all cores
    return np.ascontiguousarray(
        out.reshape(B, D, K).transpose(0, 2, 1)
    ).astype(np.float32)


kernel.last_exec_time_ns = None
kernel.last_results = None
